# revision 24
# baseline (speedup 1.0000x reference)
"""Trainium2 Bass kernel for the Actor network (MLP 64->20->50 + LSTMCell(50) + fc 50->16).

Pure data-parallel over 8 NeuronCores: batch 524288 split into 8 x 65536.

Math (per core, zeros fast path, hx=cx=0):
  h1 = relu(x @ w1.T + b1)
  h2 = relu(h1 @ w2.T + b2)
  ti = tanh((i_pre)/2), tg = tanh(g_pre), to = tanh(o_pre/2)   [0.5 folded into weights]
  u  = (1+ti)*tg        = 2*cx_new          (cx_new = sigmoid(i)*tanh(g))
  v  = (1+to)*tanh(u/2) = 2*hx_new
  out = tanh((w3/2) @ v + b3) = tanh(w3 @ hx_new + b3)
Host divides u and v by 2. Outputs computed/stored as bf16, upcast on host.

Layout: batch-major chunks of 8192 rows -> [128 partitions, 64 rows x feat]
(batch index within chunk b = p*64 + r). Feature-major intermediates via PE
transposes; two chunks (A, B) processed together, packed into partition
strips so ACT/DVE passes cover both.
"""

import numpy as np
from contextlib import ExitStack

import concourse.bass as bass
import concourse.tile as tile
from concourse import bacc, mybir
from concourse.bass_utils import run_bass_kernel_spmd
from concourse.masks import make_identity

f32 = mybir.dt.float32
bf16 = mybir.dt.bfloat16
AFT = mybir.ActivationFunctionType
ALU = mybir.AluOpType

N_CORES = 8
B_TOT = 524288
OBS, H1, H2, ACT_D = 64, 20, 50, 16
P = 128
BC = B_TOT // N_CORES      # 65536 per core
CB = 8192                  # chunk batch
R = CB // P                # 64 rows per partition
NPAIR = BC // (2 * CB)     # 4 pairs of chunks
NJ = R // 4                # 16 j-tiles per chunk (each j covers 4 rows = 512 batch)
JS = 512


# ---------------------------------------------------------------------------
# Workaround: this walrus build only accepts one sync wait per instruction on
# the TileContext exit drain. Split the global-clock waits across drains.
def _patch_tile_drain():
    from bass_rust import ScopedClock

    def _drain_and_barrier_split(self, tick_clock, wait_clock):
        drain_inst = self.nc.sync.drain()
        wait_clock.add_sem_waits(
            drain_inst.ins, ScopedClock({None: tick_clock.global_clock})
        )
        di = drain_inst.ins
        waits = list(di.sync_info.on_wait) if di.sync_info else []
        if len(waits) > 1:
            di.sync_info = mybir.SyncInfo(
                on_wait=waits[:1], on_update=list(di.sync_info.on_update)
            )
            self.nc.register_instruction(di, overwrite=True)
            for i in range(1, len(waits)):
                extra = self.nc.sync.drain().ins
                extra.sync_info = mybir.SyncInfo(on_wait=waits[i : i + 1], on_update=[])
                self.nc.register_instruction(extra, overwrite=True)

        self.nc.all_engine_barrier()
        assert self.sems is not None
        popped = self.nc._tile_sem_poison_stack.pop()
        assert popped is self._sem_poison
        self.nc.clear_and_free_semaphores(list(self.sems.allocated().values()))
        self.nc.all_engine_barrier()

    tile.TileContext._drain_and_barrier = _drain_and_barrier_split


_patch_tile_drain()


def build_zeros():
    """Fast path: hx == 0 and cx == 0."""
    nc = bacc.Bacc()
    x_e = nc.declare_dram_parameter("x", [BC, OBS], f32, isOutput=False)
    w1_e = nc.declare_dram_parameter("w1", [128, 20], f32, isOutput=False)
    w2_e = nc.declare_dram_parameter("w2", [52, 50], f32, isOutput=False)
    wg_e = nc.declare_dram_parameter("wg", [115, 150], f32, isOutput=False)
    w3_e = nc.declare_dram_parameter("w3", [50, 16], f32, isOutput=False)
    b1_e = nc.declare_dram_parameter("b1", [52, 1], f32, isOutput=False)
    b3_e = nc.declare_dram_parameter("b3", [48, 1], f32, isOutput=False)
    s2_e = nc.declare_dram_parameter("s2", [115, 1], f32, isOutput=False)
    bb2_e = nc.declare_dram_parameter("bb2", [115, 1], f32, isOutput=False)
    u_e = nc.declare_dram_parameter("u", [BC, 50], bf16, isOutput=True)
    v_e = nc.declare_dram_parameter("v", [BC, 50], bf16, isOutput=True)
    o_e = nc.declare_dram_parameter("o", [BC, 16], bf16, isOutput=True)

    with tile.TileContext(nc) as tc:
        with ExitStack() as ctx:
            const = ctx.enter_context(tc.tile_pool(name="const", bufs=1))
            xpool = ctx.enter_context(tc.tile_pool(name="xp", bufs=2))
            accp = ctx.enter_context(tc.tile_pool(name="acc", bufs=2))
            sb = ctx.enter_context(tc.tile_pool(name="sb", bufs=6))
            sb2 = ctx.enter_context(tc.tile_pool(name="sb2", bufs=8))
            ps = ctx.enter_context(tc.tile_pool(name="ps", bufs=1, space="PSUM"))
            ps2 = ctx.enter_context(tc.tile_pool(name="ps2", bufs=2, space="PSUM"))

            # constants (cast to bf16 during DMA where needed)
            w1 = const.tile([128, 20], bf16)
            nc.gpsimd.dma_start(w1[:], w1_e[:, :])
            w2 = const.tile([52, 50], bf16)
            nc.gpsimd.dma_start(w2[:], w2_e[:, :])
            wg = const.tile([115, 150], bf16)
            nc.gpsimd.dma_start(wg[:], wg_e[:, :])
            w3 = const.tile([50, 16], bf16)
            nc.gpsimd.dma_start(w3[:], w3_e[:, :])
            b1 = const.tile([52, 1], f32)
            nc.gpsimd.dma_start(b1[:], b1_e[:, :])
            b3 = const.tile([48, 1], f32)
            nc.gpsimd.dma_start(b3[:], b3_e[:, :])
            s2 = const.tile([115, 1], f32)
            nc.gpsimd.dma_start(s2[:], s2_e[:, :])
            bb2 = const.tile([115, 1], f32)
            nc.gpsimd.dma_start(bb2[:], bb2_e[:, :])
            identf = const.tile([128, 128], f32)
            make_identity(nc, identf[:])
            ident = const.tile([128, 128], bf16)
            nc.vector.tensor_copy(ident[:], identf[:])

            nchunk = BC // CB
            for q in range(nchunk):
                # ---- load chunk batch-major (cast f32->bf16), then one
                # xbar-DMA transpose: xt[p,b,a] = x_bm[a, 128b+p] which is
                # feature-major for even rows (p<64) and odd rows (p>=64).
                x_bm = xpool.tile([P, R * OBS], bf16, tag="xbm")
                nc.gpsimd.dma_start(
                    x_bm[:],
                    x_e[q * CB : (q + 1) * CB, :].rearrange(
                        "(p r) f -> p (r f)", p=P
                    ),
                )
                xt_all = xpool.tile([P, R * OBS], bf16, tag="xt")
                for xq in range(4):
                    nc.sync.dma_start_transpose(
                        xt_all[:, 1024 * xq : 1024 * xq + 1024].rearrange(
                            "p (b a) -> p b a", a=128
                        ),
                        x_bm[:, 1024 * xq : 1024 * xq + 1024],
                    )

                # pair accumulators: even-run (A) first half, odd-run (B) second
                cxp = accp.tile([P, 2 * 32 * 50], bf16, tag="cxp")
                hxp = accp.tile([P, 2 * 32 * 50], bf16, tag="hxp")
                op_ = accp.tile([P, 2 * 32 * 16], bf16, tag="op")
                # acc free layout = (block b, run c, feat): row r = 2b + c
                cxv = cxp[:].rearrange("p (b c f) -> p c b f", c=2, f=50)
                hxv = hxp[:].rearrange("p (b c f) -> p c b f", c=2, f=50)
                opv = op_[:].rearrange("p (b c f) -> p c b f", c=2, f=16)

                gabs = {}
                njc = CB // (2 * JS)  # j-tiles per chunk (8)
                for jg in range(njc // 4):
                    for j in range(4 * jg, 4 * jg + 4):
                        # ---- fc1 (A -> psum parts 0-19, B -> parts 32-51)
                        h1_ps = ps2.tile([52, JS], f32, tag="h1h2")
                        nc.tensor.matmul(
                            h1_ps[0:20, :], w1[0:64, :],
                            xt_all[0:64, JS * j : JS * j + JS],
                            start=True, stop=True, tile_position=(0, 0),
                        )
                        nc.tensor.matmul(
                            h1_ps[32:52, :], w1[64:128, :],
                            xt_all[64:128, JS * j : JS * j + JS],
                            start=True, stop=True, tile_position=(64, 32),
                        )
                        h1t = sb.tile([52, JS], bf16, tag="h1t")
                        nc.vector.tensor_scalar(
                            h1t[:], h1_ps[:], b1[:, :], 0.0, ALU.add, ALU.max
                        )

                        # ---- fc2: A (0,0) -> rows 0-49; B (32,64) -> rows 64-113
                        h2_ps = ps2.tile([115, JS], f32, tag="h1h2")
                        nc.tensor.matmul(
                            h2_ps[0:50, :], w2[0:20, :], h1t[0:20, :],
                            start=True, stop=True, tile_position=(0, 0),
                        )
                        nc.tensor.matmul(
                            h2_ps[64:114, :], w2[32:52, :], h1t[32:52, :],
                            start=True, stop=True, tile_position=(32, 64),
                        )
                        # relu2 with per-partition scale: rows 50/114 become the
                        # constant 1.0 rows consumed as the bias input of gates.
                        h2t = sb.tile([115, JS], bf16, tag="h2t")
                        nc.scalar.activation(
                            h2t[:], h2_ps[:], AFT.Relu, bias=bb2[:, :], scale=s2[:, :]
                        )

                        # ---- gates: i, g, o (K=51 incl bias row)
                        g_ps = ps.tile([128, 3 * JS], f32, tag="gps")
                        for gi in range(3):
                            nc.tensor.matmul(
                                g_ps[0:50, JS * gi : JS * gi + JS],
                                wg[0:51, 50 * gi : 50 * gi + 50],
                                h2t[0:51, :],
                                start=True, stop=True, tile_position=(0, 0),
                            )
                            nc.tensor.matmul(
                                g_ps[64:114, JS * gi : JS * gi + JS],
                                wg[64:115, 50 * gi : 50 * gi + 50],
                                h2t[64:115, :],
                                start=True, stop=True, tile_position=(64, 64),
                            )
                        tig = sb.tile([128, 3 * JS], bf16, tag="tig")
                        nc.scalar.activation(tig[:], g_ps[:], AFT.Tanh)

                        # ---- gate transposes via DMA xbar into one SBUF tile
                        # [128, (run 2, block 12, 64)]; blocks 0-3 = ti, 4-7 = tg,
                        # 8-11 = to; real data in cols 0-49 of each block
                        gab = sb2.tile([P, 1536], bf16, tag="gab")
                        gabs[j] = gab
                        nc.sync.dma_start_transpose(
                            gab[:, 0:768].rearrange("p (b a) -> p b a", a=64),
                            tig[0:64, :],
                        )
                        nc.sync.dma_start_transpose(
                            gab[:, 768:1536].rearrange("p (b a) -> p b a", a=64),
                            tig[64:128, :],
                        )
                        gv0 = gab[:].rearrange("p (c b a) -> p c b a", c=2, a=64)

                        # ---- u = (1+ti)*tg (= 2*cx_new) straight into pair acc
                        for c in range(2):
                            nc.vector.scalar_tensor_tensor(
                                cxv[:, c, 4 * j : 4 * j + 4, :],
                                gv0[:, c, 0:4, 0:50], 1.0,
                                gv0[:, c, 4:8, 0:50], ALU.add, ALU.mult,
                            )

                    # ---- tanh(u/2) batched over the 4-j group
                    tu = sb.tile([P, 1600], bf16, tag="tu")
                    nc.scalar.activation(
                        tu[:], cxp[:, 1600 * jg : 1600 * jg + 1600],
                        AFT.Tanh, scale=0.5,
                    )
                    tuv = tu[:].rearrange("p (b c f) -> p c b f", c=2, f=50)

                    for j in range(4 * jg, 4 * jg + 4):
                        gv = gabs.pop(j)[:].rearrange(
                            "p (c b a) -> p c b a", c=2, a=64
                        )
                        # ---- v = (1+to)*tanh_u (= 2*hx_new)
                        jb = 4 * (j - 4 * jg)
                        for c in range(2):
                            nc.vector.scalar_tensor_tensor(
                                hxv[:, c, 4 * j : 4 * j + 4, :],
                                gv[:, c, 8:12, 0:50], 1.0,
                                tuv[:, c, jb : jb + 4, :], ALU.add, ALU.mult,
                            )

                        # ---- vT: all inputs are full-partition (base 0), so
                        # both runs can share one bank sequentially
                        vt_ps = ps.tile([50, 2 * JS], bf16, tag="vt")
                        for t in range(4):
                            rc = (4 * j + t) * 100
                            nc.tensor.transpose(
                                vt_ps[:, 128 * t : 128 * t + 128],
                                hxp[:, rc : rc + 50], ident[:, 0:128],
                            )
                            nc.tensor.transpose(
                                vt_ps[:, JS + 128 * t : JS + 128 * t + 128],
                                hxp[:, rc + 50 : rc + 100],
                                ident[:, 0:128],
                            )
                        vt = sb.tile([50, 2 * JS], bf16, tag="vt")
                        nc.vector.tensor_copy(vt[:], vt_ps[:])
                        vtA = vt[:, 0:JS]
                        vtB = vt[:, JS : 2 * JS]

                        # ---- fc3 (w3 pre-halved): A -> parts 0-15, B -> 32-47
                        f3_ps = ps.tile([48, JS], f32, tag="f3o")
                        nc.tensor.matmul(
                            f3_ps[0:16, :], w3[:, :], vtA,
                            start=True, stop=True, tile_position=(0, 0),
                        )
                        nc.tensor.matmul(
                            f3_ps[32:48, :], w3[:, :], vtB,
                            start=True, stop=True, tile_position=(0, 32),
                        )
                        ot = sb.tile([48, JS], bf16, tag="ot")
                        nc.scalar.activation(ot[:], f3_ps[:], AFT.Tanh, bias=b3[:, :])

                        # ---- out back to batch-major via DMA xbar into acc
                        o_psA = ps.tile([128, 64], bf16, tag="f3o")
                        o_psB = ps.tile([128, 64], bf16, tag="oB")
                        for t in range(4):
                            c = 128 * t
                            nc.tensor.transpose(
                                o_psA[:, 16 * t : 16 * t + 16],
                                ot[0:16, c : c + 128], ident[0:16, 0:16],
                            )
                            nc.tensor.transpose(
                                o_psB[:, 16 * t : 16 * t + 16],
                                ot[32:48, c : c + 128], ident[32:48, 32:48],
                            )
                        nc.vector.tensor_copy(
                            opv[:, 0, 4 * j : 4 * j + 4, :],
                            o_psA[:].rearrange("p (b f) -> p b f", f=16),
                        )
                        nc.vector.tensor_copy(
                            opv[:, 1, 4 * j : 4 * j + 4, :],
                            o_psB[:].rearrange("p (b f) -> p b f", f=16),
                        )

                # ---- store chunk outputs; interleave even/odd runs on the
                # SBUF read side so the HBM write side stays contiguous
                for (acc, ext) in ((cxp, u_e), (hxp, v_e), (op_, o_e)):
                    nc.gpsimd.dma_start(
                        ext[q * CB : (q + 1) * CB, :].rearrange(
                            "(p r) f -> p (r f)", p=P
                        ),
                        acc[:],
                    )

    nc.finalize()
    return nc


def _prep_weights(fc1_w, fc1_b, fc2_w, fc2_b, w_ih, w_hh, b_ih, b_hh, fc3_w, fc3_b):
    w1h = np.zeros((128, 20), np.float32)
    w1h[0:64] = fc1_w.T
    w1h[64:128] = fc1_w.T

    w2h = np.zeros((52, 50), np.float32)
    w2h[0:20] = fc2_w.T
    w2h[32:52] = fc2_w.T

    bg = b_ih + b_hh
    wgh = np.zeros((115, 150), np.float32)
    for k, (sl, s) in enumerate(((slice(0, 50), 0.5), (slice(100, 150), 1.0), (slice(150, 200), 0.5))):
        wgh[0:50, 50 * k : 50 * k + 50] = w_ih[sl].T * s
        wgh[50, 50 * k : 50 * k + 50] = bg[sl] * s
    wgh[64:115] = wgh[0:51]

    w3h = np.ascontiguousarray((fc3_w.T * 0.5).astype(np.float32))

    b1h = np.zeros((52, 1), np.float32)
    b1h[0:20, 0] = fc1_b
    b1h[32:52, 0] = fc1_b

    b3h = np.zeros((48, 1), np.float32)
    b3h[0:16, 0] = fc3_b
    b3h[32:48, 0] = fc3_b

    s2h = np.zeros((115, 1), np.float32)
    s2h[0:50] = 1.0
    s2h[64:114] = 1.0
    bb2h = np.zeros((115, 1), np.float32)
    bb2h[0:50, 0] = fc2_b
    bb2h[64:114, 0] = fc2_b
    bb2h[50, 0] = 1.0
    bb2h[114, 0] = 1.0

    return dict(w1=w1h, w2=w2h, wg=wgh, w3=w3h, b1=b1h, b3=b3h, s2=s2h, bb2=bb2h)


_NC_CACHE = {}
TRACE = False
LAST_EXEC_NS = None


def kernel(x, hx, cx, fc1_w, fc1_b, fc2_w, fc2_b, w_ih, w_hh, b_ih, b_hh, fc3_w, fc3_b):
    x = np.asarray(x, np.float32)
    hx = np.asarray(hx, np.float32)
    cx = np.asarray(cx, np.float32)
    consts = _prep_weights(
        np.asarray(fc1_w, np.float32), np.asarray(fc1_b, np.float32),
        np.asarray(fc2_w, np.float32), np.asarray(fc2_b, np.float32),
        np.asarray(w_ih, np.float32), np.asarray(w_hh, np.float32),
        np.asarray(b_ih, np.float32), np.asarray(b_hh, np.float32),
        np.asarray(fc3_w, np.float32), np.asarray(fc3_b, np.float32),
    )
    zeros = not hx.any() and not cx.any()
    assert zeros, "general path not implemented yet"

    if "zeros" not in _NC_CACHE:
        _NC_CACHE["zeros"] = build_zeros()
    nc = _NC_CACHE["zeros"]

    in_maps = []
    for c in range(N_CORES):
        m = dict(consts)
        m["x"] = np.ascontiguousarray(x[c * BC : (c + 1) * BC])
        in_maps.append(m)

    res = run_bass_kernel_spmd(nc, in_maps, core_ids=list(range(N_CORES)), trace=TRACE)
    global LAST_EXEC_NS
    LAST_EXEC_NS = res.exec_time_ns

    out = np.empty((B_TOT, 16), np.float32)
    hx_new = np.empty((B_TOT, 50), np.float32)
    cx_new = np.empty((B_TOT, 50), np.float32)
    for c in range(N_CORES):
        r = res.results[c]
        sl = slice(c * BC, (c + 1) * BC)
        cx_new[sl] = r["u"].astype(np.float32) * 0.5
        hx_new[sl] = r["v"].astype(np.float32) * 0.5
        out[sl] = r["o"].astype(np.float32)
    return (out, hx_new, cx_new)


# revision 25
# speedup vs baseline: 1.1272x; 1.1272x over previous
"""Trainium2 Bass kernel for the Actor network (MLP 64->20->50 + LSTMCell(50) + fc 50->16).

Pure data-parallel over 8 NeuronCores: batch 524288 split into 8 x 65536.

Math (per core, zeros fast path, hx=cx=0):
  h1 = relu(x @ w1.T + b1)
  h2 = relu(h1 @ w2.T + b2)
  ti = tanh((i_pre)/2), tg = tanh(g_pre), to = tanh(o_pre/2)   [0.5 folded into weights]
  u  = (1+ti)*tg        = 2*cx_new          (cx_new = sigmoid(i)*tanh(g))
  v  = (1+to)*tanh(u/2) = 2*hx_new
  out = tanh((w3/2) @ v + b3) = tanh(w3 @ hx_new + b3)
Host divides u and v by 2. Outputs computed/stored as bf16, upcast on host.

Layout: batch-major chunks of 8192 rows -> [128 partitions, 64 rows x feat]
(batch index within chunk b = p*64 + r). Feature-major intermediates via PE
transposes; two chunks (A, B) processed together, packed into partition
strips so ACT/DVE passes cover both.
"""

import numpy as np
from contextlib import ExitStack

import concourse.bass as bass
import concourse.tile as tile
from concourse import bacc, mybir
from concourse.bass_utils import run_bass_kernel_spmd
from concourse.masks import make_identity

f32 = mybir.dt.float32
bf16 = mybir.dt.bfloat16
AFT = mybir.ActivationFunctionType
ALU = mybir.AluOpType

N_CORES = 8
B_TOT = 524288
OBS, H1, H2, ACT_D = 64, 20, 50, 16
P = 128
BC = B_TOT // N_CORES      # 65536 per core
CB = 8192                  # chunk batch
R = CB // P                # 64 rows per partition
NPAIR = BC // (2 * CB)     # 4 pairs of chunks
NJ = R // 4                # 16 j-tiles per chunk (each j covers 4 rows = 512 batch)
JS = 512


# ---------------------------------------------------------------------------
# Workaround: this walrus build only accepts one sync wait per instruction on
# the TileContext exit drain. Split the global-clock waits across drains.
def _patch_tile_drain():
    from bass_rust import ScopedClock

    def _drain_and_barrier_split(self, tick_clock, wait_clock):
        drain_inst = self.nc.sync.drain()
        wait_clock.add_sem_waits(
            drain_inst.ins, ScopedClock({None: tick_clock.global_clock})
        )
        di = drain_inst.ins
        waits = list(di.sync_info.on_wait) if di.sync_info else []
        if len(waits) > 1:
            di.sync_info = mybir.SyncInfo(
                on_wait=waits[:1], on_update=list(di.sync_info.on_update)
            )
            self.nc.register_instruction(di, overwrite=True)
            for i in range(1, len(waits)):
                extra = self.nc.sync.drain().ins
                extra.sync_info = mybir.SyncInfo(on_wait=waits[i : i + 1], on_update=[])
                self.nc.register_instruction(extra, overwrite=True)

        self.nc.all_engine_barrier()
        assert self.sems is not None
        popped = self.nc._tile_sem_poison_stack.pop()
        assert popped is self._sem_poison
        self.nc.clear_and_free_semaphores(list(self.sems.allocated().values()))
        self.nc.all_engine_barrier()

    tile.TileContext._drain_and_barrier = _drain_and_barrier_split


_patch_tile_drain()


def build_zeros():
    """Fast path: hx == 0 and cx == 0."""
    nc = bacc.Bacc()
    x_e = nc.declare_dram_parameter("x", [BC, OBS], f32, isOutput=False)
    w1_e = nc.declare_dram_parameter("w1", [128, 20], f32, isOutput=False)
    w2_e = nc.declare_dram_parameter("w2", [52, 50], f32, isOutput=False)
    wg_e = nc.declare_dram_parameter("wg", [115, 150], f32, isOutput=False)
    w3_e = nc.declare_dram_parameter("w3", [50, 16], f32, isOutput=False)
    b1_e = nc.declare_dram_parameter("b1", [52, 1], f32, isOutput=False)
    b3_e = nc.declare_dram_parameter("b3", [48, 1], f32, isOutput=False)
    s2_e = nc.declare_dram_parameter("s2", [115, 1], f32, isOutput=False)
    bb2_e = nc.declare_dram_parameter("bb2", [115, 1], f32, isOutput=False)
    u_e = nc.declare_dram_parameter("u", [BC, 50], bf16, isOutput=True)
    v_e = nc.declare_dram_parameter("v", [BC, 50], bf16, isOutput=True)
    o_e = nc.declare_dram_parameter("o", [BC, 16], bf16, isOutput=True)

    with tile.TileContext(nc) as tc:
        with ExitStack() as ctx:
            const = ctx.enter_context(tc.tile_pool(name="const", bufs=1))
            xpool = ctx.enter_context(tc.tile_pool(name="xp", bufs=2))
            accp = ctx.enter_context(tc.tile_pool(name="acc", bufs=2))
            sb = ctx.enter_context(tc.tile_pool(name="sb", bufs=6))
            sb2 = ctx.enter_context(tc.tile_pool(name="sb2", bufs=8))
            ps = ctx.enter_context(tc.tile_pool(name="ps", bufs=1, space="PSUM"))
            ps2 = ctx.enter_context(tc.tile_pool(name="ps2", bufs=2, space="PSUM"))

            # constants (cast to bf16 during DMA where needed)
            w1 = const.tile([128, 20], bf16)
            nc.gpsimd.dma_start(w1[:], w1_e[:, :])
            w2 = const.tile([52, 50], bf16)
            nc.gpsimd.dma_start(w2[:], w2_e[:, :])
            wg = const.tile([115, 150], bf16)
            nc.gpsimd.dma_start(wg[:], wg_e[:, :])
            w3 = const.tile([50, 16], bf16)
            nc.gpsimd.dma_start(w3[:], w3_e[:, :])
            b1 = const.tile([52, 1], f32)
            nc.gpsimd.dma_start(b1[:], b1_e[:, :])
            b3 = const.tile([48, 1], f32)
            nc.gpsimd.dma_start(b3[:], b3_e[:, :])
            s2 = const.tile([115, 1], f32)
            nc.gpsimd.dma_start(s2[:], s2_e[:, :])
            bb2 = const.tile([115, 1], f32)
            nc.gpsimd.dma_start(bb2[:], bb2_e[:, :])
            identf = const.tile([128, 128], f32)
            make_identity(nc, identf[:])
            ident = const.tile([128, 128], bf16)
            nc.vector.tensor_copy(ident[:], identf[:])

            nchunk = BC // CB
            for q in range(nchunk):
                # ---- load chunk batch-major (cast f32->bf16), then one
                # xbar-DMA transpose: xt[p,b,a] = x_bm[a, 128b+p] which is
                # feature-major for even rows (p<64) and odd rows (p>=64).
                x_bm = xpool.tile([P, R * OBS], bf16, tag="xbm")
                nc.gpsimd.dma_start(
                    x_bm[:],
                    x_e[q * CB : (q + 1) * CB, :].rearrange(
                        "(p r) f -> p (r f)", p=P
                    ),
                )
                xt_all = xpool.tile([P, R * OBS], bf16, tag="xt")
                for xq in range(4):
                    nc.sync.dma_start_transpose(
                        xt_all[:, 1024 * xq : 1024 * xq + 1024].rearrange(
                            "p (b a) -> p b a", a=128
                        ),
                        x_bm[:, 1024 * xq : 1024 * xq + 1024],
                    )

                # pair accumulators: even-run (A) first half, odd-run (B) second
                cxp = accp.tile([P, 2 * 32 * 50], bf16, tag="cxp")
                hxp = accp.tile([P, 2 * 32 * 50], bf16, tag="hxp")
                op_ = accp.tile([P, 2 * 32 * 16], bf16, tag="op")
                # acc free layout = (block b, run c, feat): row r = 2b + c
                cxv = cxp[:].rearrange("p (b c f) -> p c b f", c=2, f=50)
                hxv = hxp[:].rearrange("p (b c f) -> p c b f", c=2, f=50)
                opv = op_[:].rearrange("p (b c f) -> p c b f", c=2, f=16)

                gabs = {}
                njc = CB // (2 * JS)  # j-tiles per chunk (8)
                for jg in range(njc // 4):
                    for j in range(4 * jg, 4 * jg + 4):
                        # ---- fc1 (A -> psum parts 0-19, B -> parts 32-51)
                        h1_ps = ps2.tile([52, JS], f32, tag="h1h2")
                        nc.tensor.matmul(
                            h1_ps[0:20, :], w1[0:64, :],
                            xt_all[0:64, JS * j : JS * j + JS],
                            start=True, stop=True, tile_position=(0, 0),
                        )
                        nc.tensor.matmul(
                            h1_ps[32:52, :], w1[64:128, :],
                            xt_all[64:128, JS * j : JS * j + JS],
                            start=True, stop=True, tile_position=(64, 32),
                        )
                        h1t = sb.tile([52, JS], bf16, tag="h1t")
                        nc.vector.tensor_scalar(
                            h1t[:], h1_ps[:], b1[:, :], 0.0, ALU.add, ALU.max
                        )

                        # ---- fc2: A (0,0) -> rows 0-49; B (32,64) -> rows 64-113
                        h2_ps = ps2.tile([115, JS], f32, tag="h1h2")
                        nc.tensor.matmul(
                            h2_ps[0:50, :], w2[0:20, :], h1t[0:20, :],
                            start=True, stop=True, tile_position=(0, 0),
                        )
                        nc.tensor.matmul(
                            h2_ps[64:114, :], w2[32:52, :], h1t[32:52, :],
                            start=True, stop=True, tile_position=(32, 64),
                        )
                        # relu2 with per-partition scale: rows 50/114 become the
                        # constant 1.0 rows consumed as the bias input of gates.
                        h2t = sb.tile([115, JS], bf16, tag="h2t")
                        nc.scalar.activation(
                            h2t[:], h2_ps[:], AFT.Relu, bias=bb2[:, :], scale=s2[:, :]
                        )

                        # ---- gates: i, g, o (K=51 incl bias row)
                        g_ps = ps.tile([128, 3 * JS], f32, tag="gps")
                        for gi in range(3):
                            nc.tensor.matmul(
                                g_ps[0:50, JS * gi : JS * gi + JS],
                                wg[0:51, 50 * gi : 50 * gi + 50],
                                h2t[0:51, :],
                                start=True, stop=True, tile_position=(0, 0),
                            )
                            nc.tensor.matmul(
                                g_ps[64:114, JS * gi : JS * gi + JS],
                                wg[64:115, 50 * gi : 50 * gi + 50],
                                h2t[64:115, :],
                                start=True, stop=True, tile_position=(64, 64),
                            )
                        tig = sb.tile([128, 3 * JS], bf16, tag="tig")
                        nc.scalar.activation(tig[:], g_ps[:], AFT.Tanh)

                        # ---- gate transposes via DMA xbar into one SBUF tile
                        # [128, (run 2, block 12, 64)]; blocks 0-3 = ti, 4-7 = tg,
                        # 8-11 = to; real data in cols 0-49 of each block
                        gab = sb2.tile([P, 1536], bf16, tag="gab")
                        gabs[j] = gab
                        nc.sync.dma_start_transpose(
                            gab[:, 0:768].rearrange("p (b a) -> p b a", a=64),
                            tig[0:64, :],
                        )
                        nc.sync.dma_start_transpose(
                            gab[:, 768:1536].rearrange("p (b a) -> p b a", a=64),
                            tig[64:128, :],
                        )
                        gv0 = gab[:].rearrange("p (c b a) -> p c b a", c=2, a=64)

                        # ---- u = (1+ti)*tg (= 2*cx_new) straight into pair acc
                        for c in range(2):
                            nc.vector.scalar_tensor_tensor(
                                cxv[:, c, 4 * j : 4 * j + 4, :],
                                gv0[:, c, 0:4, 0:50], 1.0,
                                gv0[:, c, 4:8, 0:50], ALU.add, ALU.mult,
                            )

                    # ---- tanh(u/2) batched over the 4-j group
                    tu = sb.tile([P, 1600], bf16, tag="tu")
                    nc.scalar.activation(
                        tu[:], cxp[:, 1600 * jg : 1600 * jg + 1600],
                        AFT.Tanh, scale=0.5,
                    )
                    tuv = tu[:].rearrange("p (b c f) -> p c b f", c=2, f=50)

                    for j in range(4 * jg, 4 * jg + 4):
                        gv = gabs.pop(j)[:].rearrange(
                            "p (c b a) -> p c b a", c=2, a=64
                        )
                        # ---- v = (1+to)*tanh_u (= 2*hx_new)
                        jb = 4 * (j - 4 * jg)
                        for c in range(2):
                            nc.vector.scalar_tensor_tensor(
                                hxv[:, c, 4 * j : 4 * j + 4, :],
                                gv[:, c, 8:12, 0:50], 1.0,
                                tuv[:, c, jb : jb + 4, :], ALU.add, ALU.mult,
                            )

                        # ---- vT: all inputs are full-partition (base 0), so
                        # both runs can share one bank sequentially
                        vt_ps = ps.tile([50, 2 * JS], bf16, tag="vt")
                        for t in range(4):
                            rc = (4 * j + t) * 100
                            nc.tensor.transpose(
                                vt_ps[:, 128 * t : 128 * t + 128],
                                hxp[:, rc : rc + 50], ident[:, 0:128],
                            )
                            nc.tensor.transpose(
                                vt_ps[:, JS + 128 * t : JS + 128 * t + 128],
                                hxp[:, rc + 50 : rc + 100],
                                ident[:, 0:128],
                            )
                        vt = sb.tile([50, 2 * JS], bf16, tag="vt")
                        nc.vector.tensor_copy(vt[:], vt_ps[:])
                        vtA = vt[:, 0:JS]
                        vtB = vt[:, JS : 2 * JS]

                        # ---- fc3 (w3 pre-halved): A -> parts 0-15, B -> 32-47
                        f3_ps = ps.tile([48, JS], f32, tag="f3o")
                        nc.tensor.matmul(
                            f3_ps[0:16, :], w3[:, :], vtA,
                            start=True, stop=True, tile_position=(0, 0),
                        )
                        nc.tensor.matmul(
                            f3_ps[32:48, :], w3[:, :], vtB,
                            start=True, stop=True, tile_position=(0, 32),
                        )
                        ot = sb.tile([48, JS], bf16, tag="ot")
                        nc.scalar.activation(ot[:], f3_ps[:], AFT.Tanh, bias=b3[:, :])

                        # ---- out back to batch-major via DMA xbar into acc
                        o_psA = ps.tile([128, 64], bf16, tag="f3o")
                        o_psB = ps.tile([128, 64], bf16, tag="oB")
                        for t in range(4):
                            c = 128 * t
                            nc.tensor.transpose(
                                o_psA[:, 16 * t : 16 * t + 16],
                                ot[0:16, c : c + 128], ident[0:16, 0:16],
                            )
                            nc.tensor.transpose(
                                o_psB[:, 16 * t : 16 * t + 16],
                                ot[32:48, c : c + 128], ident[32:48, 32:48],
                            )
                        nc.vector.tensor_copy(
                            opv[:, 0, 4 * j : 4 * j + 4, :],
                            o_psA[:].rearrange("p (b f) -> p b f", f=16),
                        )
                        nc.vector.tensor_copy(
                            opv[:, 1, 4 * j : 4 * j + 4, :],
                            o_psB[:].rearrange("p (b f) -> p b f", f=16),
                        )

                # ---- store chunk outputs; interleave even/odd runs on the
                # SBUF read side so the HBM write side stays contiguous
                for (acc, ext) in ((cxp, u_e), (hxp, v_e), (op_, o_e)):
                    nc.gpsimd.dma_start(
                        ext[q * CB : (q + 1) * CB, :].rearrange(
                            "(p r) f -> p (r f)", p=P
                        ),
                        acc[:],
                    )

    nc.finalize()
    return nc


def _prep_weights(fc1_w, fc1_b, fc2_w, fc2_b, w_ih, w_hh, b_ih, b_hh, fc3_w, fc3_b):
    w1h = np.zeros((128, 20), np.float32)
    w1h[0:64] = fc1_w.T
    w1h[64:128] = fc1_w.T

    w2h = np.zeros((52, 50), np.float32)
    w2h[0:20] = fc2_w.T
    w2h[32:52] = fc2_w.T

    bg = b_ih + b_hh
    wgh = np.zeros((115, 150), np.float32)
    for k, (sl, s) in enumerate(((slice(0, 50), 0.5), (slice(100, 150), 1.0), (slice(150, 200), 0.5))):
        wgh[0:50, 50 * k : 50 * k + 50] = w_ih[sl].T * s
        wgh[50, 50 * k : 50 * k + 50] = bg[sl] * s
    wgh[64:115] = wgh[0:51]

    w3h = np.ascontiguousarray((fc3_w.T * 0.5).astype(np.float32))

    b1h = np.zeros((52, 1), np.float32)
    b1h[0:20, 0] = fc1_b
    b1h[32:52, 0] = fc1_b

    b3h = np.zeros((48, 1), np.float32)
    b3h[0:16, 0] = fc3_b
    b3h[32:48, 0] = fc3_b

    s2h = np.zeros((115, 1), np.float32)
    s2h[0:50] = 1.0
    s2h[64:114] = 1.0
    bb2h = np.zeros((115, 1), np.float32)
    bb2h[0:50, 0] = fc2_b
    bb2h[64:114, 0] = fc2_b
    bb2h[50, 0] = 1.0
    bb2h[114, 0] = 1.0

    return dict(w1=w1h, w2=w2h, wg=wgh, w3=w3h, b1=b1h, b3=b3h, s2=s2h, bb2=bb2h)


_NC_CACHE = {}
TRACE = False
LAST_EXEC_NS = None


def kernel(x, hx, cx, fc1_w, fc1_b, fc2_w, fc2_b, w_ih, w_hh, b_ih, b_hh, fc3_w, fc3_b):
    x = np.asarray(x, np.float32)
    hx = np.asarray(hx, np.float32)
    cx = np.asarray(cx, np.float32)
    consts = _prep_weights(
        np.asarray(fc1_w, np.float32), np.asarray(fc1_b, np.float32),
        np.asarray(fc2_w, np.float32), np.asarray(fc2_b, np.float32),
        np.asarray(w_ih, np.float32), np.asarray(w_hh, np.float32),
        np.asarray(b_ih, np.float32), np.asarray(b_hh, np.float32),
        np.asarray(fc3_w, np.float32), np.asarray(fc3_b, np.float32),
    )
    zeros = not hx.any() and not cx.any()
    if not zeros:
        # General fallback: run the exact reference math on the devices
        # via jax (correctness path; the spec'd inputs have hx=cx=0 and
        # take the optimized bass kernel below).
        import jax
        import jax.numpy as jnp

        def _ref(x, hx, cx, w1, b1, w2, b2, wih, whh, bih, bhh, w3, b3):
            h = jax.nn.relu(x @ w1.T + b1)
            h = jax.nn.relu(h @ w2.T + b2)
            gates = h @ wih.T + bih + hx @ whh.T + bhh
            i, f, g, o = jnp.split(gates, 4, axis=-1)
            i, f, o = jax.nn.sigmoid(i), jax.nn.sigmoid(f), jax.nn.sigmoid(o)
            g = jnp.tanh(g)
            cx_new = f * cx + i * g
            hx_new = o * jnp.tanh(cx_new)
            out = jnp.tanh(hx_new @ w3.T + b3)
            return out, hx_new, cx_new

        fn = jax.jit(_ref)
        o_, h_, c_ = fn(
            x, hx, cx,
            np.asarray(fc1_w, np.float32), np.asarray(fc1_b, np.float32),
            np.asarray(fc2_w, np.float32), np.asarray(fc2_b, np.float32),
            np.asarray(w_ih, np.float32), np.asarray(w_hh, np.float32),
            np.asarray(b_ih, np.float32), np.asarray(b_hh, np.float32),
            np.asarray(fc3_w, np.float32), np.asarray(fc3_b, np.float32),
        )
        return (np.asarray(o_), np.asarray(h_), np.asarray(c_))

    if "zeros" not in _NC_CACHE:
        _NC_CACHE["zeros"] = build_zeros()
    nc = _NC_CACHE["zeros"]

    in_maps = []
    for c in range(N_CORES):
        m = dict(consts)
        m["x"] = np.ascontiguousarray(x[c * BC : (c + 1) * BC])
        in_maps.append(m)

    res = run_bass_kernel_spmd(nc, in_maps, core_ids=list(range(N_CORES)), trace=TRACE)
    global LAST_EXEC_NS
    LAST_EXEC_NS = res.exec_time_ns

    out = np.empty((B_TOT, 16), np.float32)
    hx_new = np.empty((B_TOT, 50), np.float32)
    cx_new = np.empty((B_TOT, 50), np.float32)
    for c in range(N_CORES):
        r = res.results[c]
        sl = slice(c * BC, (c + 1) * BC)
        cx_new[sl] = r["u"].astype(np.float32) * 0.5
        hx_new[sl] = r["v"].astype(np.float32) * 0.5
        out[sl] = r["o"].astype(np.float32)
    return (out, hx_new, cx_new)
